# revision 1
# baseline (speedup 1.0000x reference)
"""Graphormer layer (pre-norm MHSA + additive attn bias + SiLU FFN) on 8 trn2 cores.

Sharding: core c handles batch b = c//4 and query rows i0 = (c%4)*512 .. +512.
Each core computes LN1 + full K/V for its batch (replicated inside the
4-core batch group), Q/scores/softmax/attn@V for its 512 query rows, the
output projection, LN2 and the full FFN for those rows.  No collectives.

Host-side prep rotates each core's token axis by -i0 so the query block is
always columns 0:512 of the same SPMD program; the attn-bias j axis is
rotated identically (softmax/attn@V are order-invariant over j).

Layouts on device are feature-major ("transposed"): xT [D, T], qT/kT [d, T],
scoresT [j, i].  The softmax denominator comes from appending a ones column
to V in the attn@V matmul; normalization uses a gpsimd partition-broadcast
of the reciprocal.  Matmul operands are bf16 (fp32 accumulation in PSUM);
the residual path stays fp32.  Softmax skips the max-subtraction: scores
are O(8) here so exp stays comfortably inside fp32 range.
"""

import sys
from contextlib import ExitStack

import numpy as np

sys.path.insert(0, "/opt/trn_rl_repo")

import ml_dtypes  # noqa: E402

import concourse.bass as bass  # noqa: E402
import concourse.bacc as bacc  # noqa: E402
import concourse.tile as tile  # noqa: E402
from concourse import mybir  # noqa: E402
from concourse.bass_utils import run_bass_kernel_spmd  # noqa: E402

F32 = mybir.dt.float32
BF16 = mybir.dt.bfloat16
AF = mybir.ActivationFunctionType
OP = mybir.AluOpType
BF16_NP = ml_dtypes.bfloat16

B, T, D = 2, 2048, 1024
H, HD = 16, 64
FF = 4 * D
N_CORES = 8
IB = 512           # query rows per core
SCALE = 1.0 / 8.0  # 1/sqrt(HD)
EPS = 1e-5

_cache = {}


def build_program():
    nc = bacc.Bacc("TRN2", target_bir_lowering=False, debug=False)

    # ---- DRAM I/O ----
    xT_d = nc.dram_tensor("xT", [D, T], F32, kind="ExternalInput").ap()
    biasT_d = nc.dram_tensor("biasT", [H, T, IB], BF16, kind="ExternalInput").ap()
    Wq_d = nc.dram_tensor("Wq", [D, D], BF16, kind="ExternalInput").ap()
    Wk_d = nc.dram_tensor("Wk", [D, D], BF16, kind="ExternalInput").ap()
    Wv_d = nc.dram_tensor("Wv", [D, D], BF16, kind="ExternalInput").ap()
    Wo_d = nc.dram_tensor("Wo", [D, D], BF16, kind="ExternalInput").ap()
    W1_d = nc.dram_tensor("W1", [D, FF], BF16, kind="ExternalInput").ap()
    W2_d = nc.dram_tensor("W2", [FF, D], BF16, kind="ExternalInput").ap()
    # packed per-partition params: [128, n_tiles] fp32
    g1_d = nc.dram_tensor("g1", [128, 8], F32, kind="ExternalInput").ap()
    bg1_d = nc.dram_tensor("bg1", [128, 8], F32, kind="ExternalInput").ap()
    g2_d = nc.dram_tensor("g2", [128, 8], F32, kind="ExternalInput").ap()
    bg2_d = nc.dram_tensor("bg2", [128, 8], F32, kind="ExternalInput").ap()
    bq8_d = nc.dram_tensor("bq8", [128, 8], F32, kind="ExternalInput").ap()
    bk_d = nc.dram_tensor("bk", [128, 8], F32, kind="ExternalInput").ap()
    bo_d = nc.dram_tensor("bo", [128, 8], F32, kind="ExternalInput").ap()
    b1_d = nc.dram_tensor("b1", [128, 32], F32, kind="ExternalInput").ap()
    b2_d = nc.dram_tensor("b2", [128, 8], F32, kind="ExternalInput").ap()
    bv_d = nc.dram_tensor("bv", [1, D], BF16, kind="ExternalInput").ap()
    outT_d = nc.dram_tensor("outT", [D, IB], F32, kind="ExternalOutput").ap()

    with tile.TileContext(nc) as tc, ExitStack() as ctx:
        # ---------------- outermost (whole-kernel lifetime) ----------------
        const_p = ctx.enter_context(tc.tile_pool(name="const", bufs=1))
        param_p = ctx.enter_context(tc.tile_pool(name="param", bufs=1))
        res_p = ctx.enter_context(tc.tile_pool(name="res", bufs=1))
        oT_p = ctx.enter_context(tc.tile_pool(name="oT", bufs=1))
        out_p = ctx.enter_context(tc.tile_pool(name="out", bufs=2))

        ones_f = const_p.tile([128, 128], F32, tag="ones_f")
        nc.vector.memset(ones_f[:], 1.0)
        ones_b = const_p.tile([1, 128], BF16, tag="ones_b")
        nc.vector.memset(ones_b[:], 1.0)
        eps_t = const_p.tile([1, 1], F32, tag="eps")
        nc.vector.memset(eps_t[:], EPS)

        def load_param(name, dram, shape, dtype=F32):
            t = param_p.tile(shape, dtype, tag=name, name=name)
            nc.sync.dma_start(t[:], dram[:])
            return t

        g1 = load_param("g1", g1_d, [128, 8])
        bg1 = load_param("bg1", bg1_d, [128, 8])
        g2 = load_param("g2", g2_d, [128, 8])
        bg2 = load_param("bg2", bg2_d, [128, 8])
        bq8 = load_param("bq8", bq8_d, [128, 8])
        bk = load_param("bk", bk_d, [128, 8])
        bo = load_param("bo", bo_d, [128, 8])
        b1 = load_param("b1", b1_d, [128, 32])
        b2 = load_param("b2", b2_d, [128, 8])
        bv = load_param("bv", bv_d, [1, D], BF16)

        # res: x residual slice in phases A-D, then reused in place as
        # xres = x + attn_out for phases D-E.
        res = [res_p.tile([128, IB], F32, tag=f"res{e}", name=f"res{e}")
               for e in range(8)]
        oT = [oT_p.tile([128, IB], BF16, tag=f"oT{d}", name=f"oT{d}")
              for d in range(8)]

        # ---------------- scope: K/V/Q (phases A-C) ------------------------
        with tc.tile_pool(name="kT", bufs=1) as kT_p, \
             tc.tile_pool(name="vcat", bufs=1) as vcat_p, \
             tc.tile_pool(name="qT", bufs=1) as qT_p:
            kT = [kT_p.tile([128, T], BF16, tag=f"kT{d}", name=f"kT{d}")
                  for d in range(8)]
            vcat = [vcat_p.tile([128, H * (HD + 1)], BF16, tag=f"vc{t}",
                                name=f"vc{t}") for t in range(16)]
            qT = [qT_p.tile([128, IB], BF16, tag=f"qT{d}", name=f"qT{d}")
                  for d in range(8)]

            # ---------------- scope: hT (phases A-B) -----------------------
            with tc.tile_pool(name="hT", bufs=1) as hT_p:
                hT = [hT_p.tile([128, T], BF16, tag=f"hT{e}", name=f"hT{e}")
                      for e in range(8)]

                # ===== Phase A: LN1 (streamed, partition-axis stats) =======
                with tc.tile_pool(name="xc", bufs=2) as xc_p, \
                     tc.tile_pool(name="sq", bufs=3) as sq_p, \
                     tc.tile_pool(name="lnt", bufs=2) as lnt_p, \
                     tc.tile_pool(name="lnb", bufs=2) as lnb_p, \
                     tc.tile_pool(name="lnps", bufs=2,
                                  space=bass.MemorySpace.PSUM) as lnps_p:
                    for n in range(4):
                        nb = slice(n * 512, (n + 1) * 512)
                        xcs = []
                        ps_mu = lnps_p.tile([1, 512], F32, tag="psmu", name="psmu")
                        ps_sq = lnps_p.tile([1, 512], F32, tag="pssq", name="pssq")
                        for e in range(8):
                            xc = xc_p.tile([128, 512], F32, tag=f"xc{e}", name="xc")
                            nc.sync.dma_start(xc[:], xT_d[e * 128:(e + 1) * 128, nb])
                            xcs.append(xc)
                            nc.tensor.matmul(ps_mu[:], ones_f[:, 0:1], xc[:],
                                             start=(e == 0), stop=(e == 7))
                            x2 = sq_p.tile([128, 512], F32, tag="x2", name="x2")
                            nc.scalar.square(x2[:], xc[:])
                            nc.tensor.matmul(ps_sq[:], ones_f[:, 0:1], x2[:],
                                             start=(e == 0), stop=(e == 7))
                        mu_n = lnt_p.tile([1, 512], F32, tag="mu_n", name="mu_n")
                        nc.scalar.activation(mu_n[:], ps_mu[:], AF.Identity,
                                             scale=1.0 / D)
                        mu2_n = lnt_p.tile([1, 512], F32, tag="mu2_n", name="mu2_n")
                        nc.scalar.square(mu2_n[:], mu_n[:])
                        var_n = lnt_p.tile([1, 512], F32, tag="var_n", name="var_n")
                        nc.vector.scalar_tensor_tensor(
                            var_n[:], ps_sq[:], 1.0 / D, mu2_n[:],
                            op0=OP.mult, op1=OP.subtract)
                        std_n = lnt_p.tile([1, 512], F32, tag="std_n", name="std_n")
                        nc.scalar.activation(std_n[:], var_n[:], AF.Sqrt, bias=eps_t[:])
                        rstd_n = lnt_p.tile([1, 512], F32, tag="rstd_n", name="rstd_n")
                        nc.vector.reciprocal(rstd_n[:], std_n[:])
                        mu_b = lnb_p.tile([128, 512], F32, tag="mu_b", name="mu_b")
                        nc.gpsimd.partition_broadcast(mu_b[:], mu_n[:])
                        rstd_b = lnb_p.tile([128, 512], F32, tag="rstd_b",
                                            name="rstd_b")
                        nc.gpsimd.partition_broadcast(rstd_b[:], rstd_n[:])
                        for e in range(8):
                            if n == 0:
                                nc.scalar.activation(res[e][:], xcs[e][:],
                                                     AF.Identity)
                            t = sq_p.tile([128, 512], F32, tag="lnap", name="lnap")
                            nc.vector.tensor_sub(t[:], xcs[e][:], mu_b[:])
                            nc.vector.tensor_mul(t[:], t[:], rstd_b[:])
                            nc.scalar.activation(hT[e][:, nb], t[:], AF.Identity,
                                                 scale=g1[:, e:e + 1],
                                                 bias=bg1[:, e:e + 1])

                # ===== Phase B: Q/K/V projections ==========================
                with tc.tile_pool(name="wp", bufs=12) as wp, \
                     tc.tile_pool(name="wv512", bufs=2) as wv_p, \
                     tc.tile_pool(name="pps", bufs=4,
                                  space=bass.MemorySpace.PSUM) as pps:
                    # qT[d, i] for this core's rows (= token cols 0:IB)
                    for dt in range(8):
                        ps = pps.tile([128, 512], F32, tag="ps", name="psq")
                        for e in range(8):
                            wt = wp.tile([128, 128], BF16, tag="w", name="wq")
                            nc.sync.dma_start(
                                wt[:], Wq_d[e * 128:(e + 1) * 128,
                                            dt * 128:(dt + 1) * 128])
                            nc.tensor.matmul(ps[:], wt[:], hT[e][:, 0:IB],
                                             start=(e == 0), stop=(e == 7))
                        nc.scalar.activation(qT[dt][:], ps[:], AF.Identity,
                                             scale=SCALE, bias=bq8[:, dt:dt + 1])
                    # kT[d, j] over all tokens
                    for dt in range(8):
                        for n in range(4):
                            nb = slice(n * 512, (n + 1) * 512)
                            ps = pps.tile([128, 512], F32, tag="ps", name="psk")
                            for e in range(8):
                                wt = wp.tile([128, 128], BF16, tag="w", name="wk")
                                nc.sync.dma_start(
                                    wt[:], Wk_d[e * 128:(e + 1) * 128,
                                                dt * 128:(dt + 1) * 128])
                                nc.tensor.matmul(ps[:], wt[:], hT[e][:, nb],
                                                 start=(e == 0), stop=(e == 7))
                            nc.scalar.activation(kT[dt][:, nb], ps[:], AF.Identity,
                                                 bias=bk[:, dt:dt + 1])
                    # v[j, d] natural layout + ones column per head
                    for tt in range(16):
                        nc.vector.memset(
                            vcat[tt][:].rearrange(
                                "p (h x) -> p h x", x=HD + 1)[:, :, HD:HD + 1],
                            1.0)
                    for n in range(2):
                        nb = slice(n * 512, (n + 1) * 512)
                        wv_tiles = []
                        for e in range(8):
                            wv = wv_p.tile([128, 512], BF16, tag=f"wv{e}",
                                           name=f"wv{e}")
                            nc.sync.dma_start(wv[:], Wv_d[e * 128:(e + 1) * 128, nb])
                            wv_tiles.append(wv)
                        for tt in range(16):
                            tb = slice(tt * 128, (tt + 1) * 128)
                            ps = pps.tile([128, 512], F32, tag="ps", name="psv")
                            for e in range(8):
                                nc.tensor.matmul(ps[:], hT[e][:, tb],
                                                 wv_tiles[e][:],
                                                 start=(e == 0), stop=False)
                            nc.tensor.matmul(ps[:], ones_b[:], bv[:, nb],
                                             start=False, stop=True)
                            dst = vcat[tt][:, n * 8 * (HD + 1):(n + 1) * 8 * (HD + 1)]
                            dst = dst.rearrange("p (h x) -> p h x",
                                                x=HD + 1)[:, :, 0:HD]
                            src = ps[:].rearrange("p (h d) -> p h d", d=HD)
                            nc.scalar.activation(dst, src, AF.Identity)
            # hT pool closed here

            # ===== Phase C: attention ======================================
            with tc.tile_pool(name="biasdma", bufs=8) as bias_p, \
                 tc.tile_pool(name="upre", bufs=4) as up_p, \
                 tc.tile_pool(name="uexp", bufs=4) as u_p, \
                 tc.tile_pool(name="nrm", bufs=2) as nrm_p, \
                 tc.tile_pool(name="pss", bufs=2,
                              space=bass.MemorySpace.PSUM) as pss, \
                 tc.tile_pool(name="pso", bufs=2,
                              space=bass.MemorySpace.PSUM) as pso:
                for h in range(H):
                    dt, po = h // 2, (h % 2) * 64
                    ps_o = pso.tile([HD + 1, 512], F32, tag="ps_o", name="ps_o")
                    for j in range(16):
                        jb = slice(j * 128, (j + 1) * 128)
                        ps_s = pss.tile([128, 512], F32, tag="ps_s", name="ps_s")
                        nc.tensor.matmul(ps_s[:], kT[dt][po:po + 64, jb],
                                         qT[dt][po:po + 64, :],
                                         start=True, stop=True)
                        bt = bias_p.tile([128, IB], BF16, tag="bt", name="bt")
                        nc.sync.dma_start(bt[:], biasT_d[h, jb, :])
                        up = up_p.tile([128, IB], F32, tag="up", name="up")
                        nc.vector.scalar_tensor_tensor(up[:], ps_s[:], 1.0, bt[:],
                                                       op0=OP.mult, op1=OP.add)
                        u = u_p.tile([128, IB], BF16, tag="u", name="u")
                        nc.scalar.activation(u[:], up[:], AF.Exp)
                        nc.tensor.matmul(
                            ps_o[:], vcat[j][:, h * (HD + 1):(h + 1) * (HD + 1)],
                            u[:], start=(j == 0), stop=(j == 15))
                    recip = nrm_p.tile([1, 512], F32, tag="recip", name="recip")
                    nc.vector.reciprocal(recip[:], ps_o[64:65, :])
                    rb = nrm_p.tile([64, 512], F32, tag="rb", name="rb")
                    nc.gpsimd.partition_broadcast(rb[:], recip[:])
                    nc.vector.tensor_mul(oT[dt][po:po + 64, :], ps_o[0:64, :],
                                         rb[:])
        # kT/vcat/qT pools closed here

        # ---------------- scope: h2/sz (phases D-E) ------------------------
        with tc.tile_pool(name="h2", bufs=1) as h2_p, \
             tc.tile_pool(name="sz", bufs=1) as sz_p:
            h2 = [h2_p.tile([128, IB], BF16, tag=f"h2{e}", name=f"h2{e}")
                  for e in range(8)]
            sz = [sz_p.tile([128, IB], BF16, tag=f"sz{f}", name=f"sz{f}")
                  for f in range(32)]

            # ===== Phase D: out-projection + LN2 ===========================
            with tc.tile_pool(name="wp2", bufs=12) as wp2, \
                 tc.tile_pool(name="sq2", bufs=3) as sq2_p, \
                 tc.tile_pool(name="lnt2", bufs=2) as lnt2_p, \
                 tc.tile_pool(name="lnb2", bufs=2) as lnb2_p, \
                 tc.tile_pool(name="dps", bufs=2,
                              space=bass.MemorySpace.PSUM) as dps, \
                 tc.tile_pool(name="dps1", bufs=1,
                              space=bass.MemorySpace.PSUM) as dps1:
                for et in range(8):
                    ps = dps.tile([128, 512], F32, tag="psx1", name="psx1")
                    for dt in range(8):
                        wt = wp2.tile([128, 128], BF16, tag="w2", name="wo")
                        nc.sync.dma_start(wt[:], Wo_d[dt * 128:(dt + 1) * 128,
                                                      et * 128:(et + 1) * 128])
                        nc.tensor.matmul(ps[:], wt[:], oT[dt][:],
                                         start=(dt == 0), stop=(dt == 7))
                    # res[et] <- x + attn_out (+bo), in place
                    nc.vector.scalar_tensor_tensor(res[et][:], ps[:],
                                                   bo[:, et:et + 1], res[et][:],
                                                   op0=OP.add, op1=OP.add)
                # LN2 (single 512-col block)
                ps_mu = dps1.tile([1, 512], F32, tag="psmu2", name="psmu2")
                for e in range(8):
                    nc.tensor.matmul(ps_mu[:], ones_f[:, 0:1], res[e][:],
                                     start=(e == 0), stop=(e == 7))
                ps_sq = dps1.tile([1, 512], F32, tag="pssq2", name="pssq2")
                for e in range(8):
                    x2 = sq2_p.tile([128, 512], F32, tag="x22", name="x22")
                    nc.scalar.square(x2[:], res[e][:])
                    nc.tensor.matmul(ps_sq[:], ones_f[:, 0:1], x2[:],
                                     start=(e == 0), stop=(e == 7))
                mu_n = lnt2_p.tile([1, 512], F32, tag="mu_n2", name="mu_n2")
                nc.scalar.activation(mu_n[:], ps_mu[:], AF.Identity, scale=1.0 / D)
                mu2_n = lnt2_p.tile([1, 512], F32, tag="mu2_n2", name="mu2_n2")
                nc.scalar.square(mu2_n[:], mu_n[:])
                var_n = lnt2_p.tile([1, 512], F32, tag="var_n2", name="var_n2")
                nc.vector.scalar_tensor_tensor(var_n[:], ps_sq[:], 1.0 / D,
                                               mu2_n[:], op0=OP.mult,
                                               op1=OP.subtract)
                std_n = lnt2_p.tile([1, 512], F32, tag="std_n2", name="std_n2")
                nc.scalar.activation(std_n[:], var_n[:], AF.Sqrt, bias=eps_t[:])
                rstd_n = lnt2_p.tile([1, 512], F32, tag="rstd_n2", name="rstd_n2")
                nc.vector.reciprocal(rstd_n[:], std_n[:])
                mu_b = lnb2_p.tile([128, 512], F32, tag="mu_b2", name="mu_b2")
                nc.gpsimd.partition_broadcast(mu_b[:], mu_n[:])
                rstd_b = lnb2_p.tile([128, 512], F32, tag="rstd_b2",
                                     name="rstd_b2")
                nc.gpsimd.partition_broadcast(rstd_b[:], rstd_n[:])
                for e in range(8):
                    t = sq2_p.tile([128, IB], F32, tag="lnap2", name="lnap2")
                    nc.vector.tensor_sub(t[:], res[e][:], mu_b[:])
                    nc.vector.tensor_mul(t[:], t[:], rstd_b[:])
                    nc.scalar.activation(h2[e][:], t[:], AF.Identity,
                                         scale=g2[:, e:e + 1],
                                         bias=bg2[:, e:e + 1])

            # ===== Phase E: FFN ============================================
            with tc.tile_pool(name="wp3", bufs=16) as wp3, \
                 tc.tile_pool(name="sg", bufs=3) as sg_p, \
                 tc.tile_pool(name="eps", bufs=4,
                              space=bass.MemorySpace.PSUM) as eps_p:
                for ft in range(32):
                    ps = eps_p.tile([128, 512], F32, tag="pse", name="psz")
                    for e in range(8):
                        wt = wp3.tile([128, 128], BF16, tag="w3", name="w1t")
                        nc.sync.dma_start(wt[:], W1_d[e * 128:(e + 1) * 128,
                                                      ft * 128:(ft + 1) * 128])
                        nc.tensor.matmul(ps[:], wt[:], h2[e][:],
                                         start=(e == 0), stop=(e == 7))
                    sg = sg_p.tile([128, IB], BF16, tag="sg", name="sg")
                    nc.scalar.activation(sg[:], ps[:], AF.Sigmoid,
                                         bias=b1[:, ft:ft + 1])
                    # silu(z) = z * sigmoid(z), z = ps + b1
                    nc.vector.scalar_tensor_tensor(sz[ft][:], ps[:],
                                                   b1[:, ft:ft + 1], sg[:],
                                                   op0=OP.add, op1=OP.mult)
                for et in range(8):
                    ps = eps_p.tile([128, 512], F32, tag="pse", name="psy")
                    for ft in range(32):
                        wt = wp3.tile([128, 128], BF16, tag="w3", name="w2t")
                        nc.sync.dma_start(wt[:], W2_d[ft * 128:(ft + 1) * 128,
                                                      et * 128:(et + 1) * 128])
                        nc.tensor.matmul(ps[:], wt[:], sz[ft][:],
                                         start=(ft == 0), stop=(ft == 31))
                    ot = out_p.tile([128, IB], F32, tag="outt", name="outt")
                    nc.vector.scalar_tensor_tensor(ot[:], ps[:], b2[:, et:et + 1],
                                                   res[et][:], op0=OP.add,
                                                   op1=OP.add)
                    nc.sync.dma_start(outT_d[et * 128:(et + 1) * 128, :], ot[:])

    nc.compile()
    return nc


def _prep_inputs(inputs):
    """Host-side layout prep -> list of 8 per-core input maps."""
    x = np.asarray(inputs["x"], dtype=np.float32)
    ab = np.asarray(inputs["attn_bias"], dtype=np.float32)

    def pack(v, ntiles):
        return np.ascontiguousarray(
            np.asarray(v, np.float32).reshape(ntiles, 128).T)

    shared = {
        "Wq": np.ascontiguousarray(np.asarray(inputs["Wq"]).astype(BF16_NP)),
        "Wk": np.ascontiguousarray(np.asarray(inputs["Wk"]).astype(BF16_NP)),
        "Wv": np.ascontiguousarray(np.asarray(inputs["Wv"]).astype(BF16_NP)),
        "Wo": np.ascontiguousarray(np.asarray(inputs["Wo"]).astype(BF16_NP)),
        "W1": np.ascontiguousarray(np.asarray(inputs["W1"]).astype(BF16_NP)),
        "W2": np.ascontiguousarray(np.asarray(inputs["W2"]).astype(BF16_NP)),
        "g1": pack(inputs["ln1_g"], 8),
        "bg1": pack(inputs["ln1_b"], 8),
        "g2": pack(inputs["ln2_g"], 8),
        "bg2": pack(inputs["ln2_b"], 8),
        "bq8": pack(np.asarray(inputs["bq"], np.float32) * SCALE, 8),
        "bk": pack(inputs["bk"], 8),
        "bo": pack(inputs["bo"], 8),
        "b1": pack(inputs["b1"], 32),
        "b2": pack(inputs["b2"], 8),
        "bv": np.ascontiguousarray(
            np.asarray(inputs["bv"], np.float32).astype(BF16_NP).reshape(1, D)),
    }
    xT = [np.ascontiguousarray(x[b].T) for b in range(B)]  # [D, T] f32
    ab_bf = ab.astype(BF16_NP)  # [B, H, T(i), T(j)]
    in_maps = []
    for c in range(N_CORES):
        b, i0 = c // 4, (c % 4) * IB
        # token axis rotated by -i0 (queries land at cols 0:IB); the j axis
        # of the bias is rotated identically to match k/v token order.
        xTc = np.ascontiguousarray(np.roll(xT[b], -i0, axis=1))
        biasT = np.ascontiguousarray(
            np.roll(ab_bf[b, :, i0:i0 + IB, :], -i0, axis=2).transpose(0, 2, 1))
        m = {"xT": xTc, "biasT": biasT}
        m.update(shared)
        in_maps.append(m)
    return in_maps


def kernel(**inputs):
    if "nc" not in _cache:
        _cache["nc"] = build_program()
    nc = _cache["nc"]
    in_maps = _prep_inputs(inputs)
    r = run_bass_kernel_spmd(nc, in_maps, list(range(N_CORES)))
    out = np.empty((B, T, D), dtype=np.float32)
    for c in range(N_CORES):
        b, i0 = c // 4, (c % 4) * IB
        out[b, i0:i0 + IB, :] = np.asarray(r.results[c]["outT"], np.float32).T
    return out



# revision 2
# speedup vs baseline: 1.1598x; 1.1598x over previous
"""Graphormer layer (pre-norm MHSA + additive attn bias + SiLU FFN) on 8 trn2 cores.

Sharding: core c handles batch b = c//4 and query rows i0 = (c%4)*512 .. +512.
Each core computes LN1 + full K/V for its batch (replicated inside the
4-core batch group), Q/scores/softmax/attn@V for its 512 query rows, the
output projection, LN2 and the full FFN for those rows.  No collectives.

v2 changes vs baseline (which was DMA-dispatch-bound at ~600ns/dma on
SP.SEQ + HWDGE):
  - ~50 large DMAs instead of ~1200 small ones: weights host-packed into
    [128, N] row-block layouts loaded in 1 DMA each; attn bias host-packed
    per head as [128, 16*512] tiles (16 DMAs).
  - host precomputes exp(bias); device computes u = exp(scores) straight
    from PSUM on ACT, then one fast bf16 multiply u*exp(bias) on DVE
    (replaces the f32 bias-add + exp roundtrip).
  - bv folded into bo on host (softmax rows sum to 1 so (v+bv)@Wo =
    v@Wo + bv@Wo), removing the ones-row matmuls in the V projection.
  - loop orders reuse the matmul stationary operand (K proj: 1 ldweights
    per 4 matmuls) and keep PSUM accumulation groups within 8 banks.

Layouts on device are feature-major ("transposed"): xT [D, T], qT/kT [d, T],
scoresT [j, i].  The softmax denominator comes from appending a ones column
to V in the attn@V matmul; normalization uses a gpsimd partition-broadcast
of the reciprocal.  Matmul operands are bf16 (fp32 accumulation in PSUM);
the residual path stays fp32.  Softmax skips the max-subtraction: scores
are O(8) here so exp stays comfortably inside fp32/bf16 range.
"""

import sys
from contextlib import ExitStack

import numpy as np

sys.path.insert(0, "/opt/trn_rl_repo")

import ml_dtypes  # noqa: E402

import concourse.bass as bass  # noqa: E402
import concourse.bacc as bacc  # noqa: E402
import concourse.tile as tile  # noqa: E402
from concourse import mybir  # noqa: E402
from concourse.bass_utils import run_bass_kernel_spmd  # noqa: E402

F32 = mybir.dt.float32
BF16 = mybir.dt.bfloat16
AF = mybir.ActivationFunctionType
OP = mybir.AluOpType
BF16_NP = ml_dtypes.bfloat16

B, T, D = 2, 2048, 1024
H, HD = 16, 64
FF = 4 * D
N_CORES = 8
IB = 512           # query rows per core
SCALE = 1.0 / 8.0  # 1/sqrt(HD)
EPS = 1e-5

_cache = {}


def build_program():
    nc = bacc.Bacc("TRN2", target_bir_lowering=False, debug=False)

    # ---- DRAM I/O ----
    xT_d = nc.dram_tensor("xT", [D, T], F32, kind="ExternalInput").ap()
    # exp(bias) per head: [H, 128, 16*512]; [h, p, jb*512+i] = exp(b[h, i, jb*128+p])
    expb_d = nc.dram_tensor("expb", [H, 128, 16 * IB], BF16, kind="ExternalInput").ap()
    # weights packed as [128, ...] row-block layouts (see _prep_inputs)
    wqkv_d = nc.dram_tensor("wqkv", [128, 3 * 8 * D], BF16, kind="ExternalInput").ap()
    wo_d = nc.dram_tensor("wo", [128, 8 * D], BF16, kind="ExternalInput").ap()
    w1_d = nc.dram_tensor("w1", [128, 8 * FF], BF16, kind="ExternalInput").ap()
    w2_d = nc.dram_tensor("w2", [128, 32 * D], BF16, kind="ExternalInput").ap()
    # packed per-partition params: [128, n_tiles] fp32
    g1_d = nc.dram_tensor("g1", [128, 8], F32, kind="ExternalInput").ap()
    bg1_d = nc.dram_tensor("bg1", [128, 8], F32, kind="ExternalInput").ap()
    g2_d = nc.dram_tensor("g2", [128, 8], F32, kind="ExternalInput").ap()
    bg2_d = nc.dram_tensor("bg2", [128, 8], F32, kind="ExternalInput").ap()
    bq8_d = nc.dram_tensor("bq8", [128, 8], F32, kind="ExternalInput").ap()
    bk_d = nc.dram_tensor("bk", [128, 8], F32, kind="ExternalInput").ap()
    bo2_d = nc.dram_tensor("bo2", [128, 8], F32, kind="ExternalInput").ap()
    b1_d = nc.dram_tensor("b1", [128, 32], F32, kind="ExternalInput").ap()
    b2_d = nc.dram_tensor("b2", [128, 8], F32, kind="ExternalInput").ap()
    outT_d = nc.dram_tensor("outT", [D, IB], F32, kind="ExternalOutput").ap()

    with tile.TileContext(nc) as tc, ExitStack() as ctx:
        # ---------------- outermost (whole-kernel lifetime) ----------------
        const_p = ctx.enter_context(tc.tile_pool(name="const", bufs=1))
        param_p = ctx.enter_context(tc.tile_pool(name="param", bufs=1))
        out_p = ctx.enter_context(tc.tile_pool(name="out", bufs=2))

        ones_f = const_p.tile([128, 1], F32, tag="ones_f")
        nc.vector.memset(ones_f[:], 1.0)
        eps_t = const_p.tile([1, 1], F32, tag="eps")
        nc.vector.memset(eps_t[:], EPS)

        def load_param(name, dram, shape, dtype=F32):
            t = param_p.tile(shape, dtype, tag=name, name=name)
            nc.sync.dma_start(t[:], dram[:])
            return t

        g1 = load_param("g1", g1_d, [128, 8])
        bg1 = load_param("bg1", bg1_d, [128, 8])
        g2 = load_param("g2", g2_d, [128, 8])
        bg2 = load_param("bg2", bg2_d, [128, 8])
        bq8 = load_param("bq8", bq8_d, [128, 8])
        bk = load_param("bk", bk_d, [128, 8])
        bo2 = load_param("bo2", bo2_d, [128, 8])
        b1 = load_param("b1", b1_d, [128, 32])
        b2 = load_param("b2", b2_d, [128, 8])

        # ---------------- scope: oT (phases C-D) ---------------------------
        with tc.tile_pool(name="oT", bufs=1) as oT_p:
            oT = [oT_p.tile([128, IB], BF16, tag=f"oT{d}", name=f"oT{d}")
                  for d in range(8)]

            # ------------- scope: K/V/Q (phases A-C) -----------------------
            with tc.tile_pool(name="kT", bufs=1) as kT_p, \
                 tc.tile_pool(name="vcat", bufs=1) as vcat_p, \
                 tc.tile_pool(name="qT", bufs=1) as qT_p:
                kT = [kT_p.tile([128, T], BF16, tag=f"kT{d}", name=f"kT{d}")
                      for d in range(8)]
                vcat = [vcat_p.tile([128, H * (HD + 1)], BF16, tag=f"vc{t}",
                                    name=f"vc{t}") for t in range(16)]
                qT = [qT_p.tile([128, IB], BF16, tag=f"qT{d}", name=f"qT{d}")
                      for d in range(8)]

                # ------------- scope: hT + wqkv (phases A-B) ---------------
                with tc.tile_pool(name="hT", bufs=1) as hT_p, \
                     tc.tile_pool(name="wqk", bufs=1) as wqk_p:
                    hT = [hT_p.tile([128, T], BF16, tag=f"hT{e}", name=f"hT{e}")
                          for e in range(8)]
                    wqk = wqk_p.tile([128, 2 * 8 * D], BF16, tag="wqk",
                                     name="wqk")
                    nc.sync.dma_start(wqk[:], wqkv_d[:, 0:2 * 8 * D])

                    # ===== Phase A: LN1 (streamed, partition-axis stats) ===
                    with tc.tile_pool(name="xc", bufs=10) as xc_p, \
                         tc.tile_pool(name="sq", bufs=2) as sq_p, \
                         tc.tile_pool(name="lnt", bufs=6) as lnt_p, \
                         tc.tile_pool(name="lnb", bufs=2) as lnb_p, \
                         tc.tile_pool(name="lnps", bufs=4,
                                      space=bass.MemorySpace.PSUM) as lnps_p:
                        for n in range(4):
                            nb = slice(n * 512, (n + 1) * 512)
                            xcs = []
                            for e in range(8):
                                xc = xc_p.tile([128, 512], F32, tag="xc",
                                               name="xc")
                                nc.sync.dma_start(
                                    xc[:], xT_d[e * 128:(e + 1) * 128, nb])
                                xcs.append(xc)
                            ps_mu = lnps_p.tile([1, 512], F32, tag="psmu",
                                                name="psmu")
                            ps_sq = lnps_p.tile([1, 512], F32, tag="pssq",
                                                name="pssq")
                            for e in range(8):
                                nc.tensor.matmul(ps_mu[:], ones_f[:],
                                                 xcs[e][:],
                                                 start=(e == 0), stop=(e == 7))
                            for e in range(8):
                                x2 = sq_p.tile([128, 512], F32, tag="x2",
                                               name="x2")
                                nc.vector.tensor_mul(x2[:], xcs[e][:],
                                                     xcs[e][:])
                                nc.tensor.matmul(ps_sq[:], ones_f[:], x2[:],
                                                 start=(e == 0), stop=(e == 7))
                            mu_n = lnt_p.tile([1, 512], F32, tag="stat",
                                              name="mu_n")
                            nc.scalar.activation(mu_n[:], ps_mu[:], AF.Identity,
                                                 scale=1.0 / D)
                            mu2_n = lnt_p.tile([1, 512], F32, tag="stat",
                                               name="mu2_n")
                            nc.scalar.square(mu2_n[:], mu_n[:])
                            var_n = lnt_p.tile([1, 512], F32, tag="stat",
                                               name="var_n")
                            nc.vector.scalar_tensor_tensor(
                                var_n[:], ps_sq[:], 1.0 / D, mu2_n[:],
                                op0=OP.mult, op1=OP.subtract)
                            std_n = lnt_p.tile([1, 512], F32, tag="stat",
                                               name="std_n")
                            nc.scalar.activation(std_n[:], var_n[:], AF.Sqrt,
                                                 bias=eps_t[:])
                            rstd_n = lnt_p.tile([1, 512], F32, tag="stat",
                                                name="rstd_n")
                            nc.vector.reciprocal(rstd_n[:], std_n[:])
                            mu_b = lnb_p.tile([128, 512], F32, tag="mu_b",
                                              name="mu_b")
                            nc.gpsimd.partition_broadcast(mu_b[:], mu_n[:])
                            rstd_b = lnb_p.tile([128, 512], F32, tag="rstd_b",
                                                name="rstd_b")
                            nc.gpsimd.partition_broadcast(rstd_b[:], rstd_n[:])
                            for e in range(8):
                                t = sq_p.tile([128, 512], F32, tag="lnap",
                                              name="lnap")
                                nc.vector.tensor_sub(t[:], xcs[e][:], mu_b[:])
                                nc.vector.tensor_mul(t[:], t[:], rstd_b[:])
                                nc.vector.tensor_scalar(
                                    hT[e][:, nb], t[:], g1[:, e:e + 1],
                                    bg1[:, e:e + 1], op0=OP.mult, op1=OP.add)

                    # ===== Phase B: K/Q/V projections ======================
                    with tc.tile_pool(name="wv", bufs=1) as wv_p, \
                         tc.tile_pool(name="pps", bufs=8,
                                      space=bass.MemorySpace.PSUM) as pps:
                        wv = wv_p.tile([128, 8 * D], BF16, tag="wv", name="wv")
                        nc.sync.dma_start(wv[:],
                                          wqkv_d[:, 2 * 8 * D:3 * 8 * D])
                        # kT[d, j] over all tokens; stationary reused over n
                        for dt in range(8):
                            pk = [pps.tile([128, 512], F32, tag="ps", name="psk")
                                  for _ in range(4)]
                            for e in range(8):
                                wsl = wqk[:, 8 * D + e * D + dt * 128:
                                          8 * D + e * D + (dt + 1) * 128]
                                for n in range(4):
                                    nc.tensor.matmul(
                                        pk[n][:], wsl,
                                        hT[e][:, n * 512:(n + 1) * 512],
                                        start=(e == 0), stop=(e == 7))
                            for n in range(4):
                                nc.vector.tensor_scalar_add(
                                    kT[dt][:, n * 512:(n + 1) * 512], pk[n][:],
                                    bk[:, dt:dt + 1])
                        # qT[d, i] for this core's rows (= token cols 0:IB)
                        for dt in range(8):
                            ps = pps.tile([128, 512], F32, tag="ps", name="psq")
                            for e in range(8):
                                nc.tensor.matmul(
                                    ps[:],
                                    wqk[:, e * D + dt * 128:e * D + (dt + 1) * 128],
                                    hT[e][:, 0:IB], start=(e == 0), stop=(e == 7))
                            nc.scalar.activation(qT[dt][:], ps[:], AF.Identity,
                                                 scale=SCALE,
                                                 bias=bq8[:, dt:dt + 1])
                        # v[j, d] natural layout + ones column per head
                        for tt in range(16):
                            nc.vector.memset(
                                vcat[tt][:].rearrange(
                                    "p (h x) -> p h x", x=HD + 1)[:, :,
                                                                  HD:HD + 1],
                                1.0)
                        for tt in range(16):
                            tb = slice(tt * 128, (tt + 1) * 128)
                            pv = [pps.tile([128, 512], F32, tag="ps", name="psv")
                                  for _ in range(2)]
                            for e in range(8):
                                for n in range(2):
                                    nc.tensor.matmul(
                                        pv[n][:], hT[e][:, tb],
                                        wv[:, e * D + n * 512:
                                           e * D + (n + 1) * 512],
                                        start=(e == 0), stop=(e == 7))
                            for n in range(2):
                                dst = vcat[tt][:, n * 8 * (HD + 1):
                                               (n + 1) * 8 * (HD + 1)]
                                dst = dst.rearrange("p (h x) -> p h x",
                                                    x=HD + 1)[:, :, 0:HD]
                                src = pv[n][:].rearrange("p (h d) -> p h d",
                                                         d=HD)
                                nc.scalar.activation(dst, src, AF.Identity)
                # hT/wqkv pools closed here

                # ===== Phase C: attention ==================================
                with tc.tile_pool(name="ebias", bufs=3) as eb_p, \
                     tc.tile_pool(name="uexp", bufs=4) as u_p, \
                     tc.tile_pool(name="uexb", bufs=4) as ue_p, \
                     tc.tile_pool(name="nrm", bufs=2) as nrm_p, \
                     tc.tile_pool(name="pss", bufs=3,
                                  space=bass.MemorySpace.PSUM) as pss, \
                     tc.tile_pool(name="pso", bufs=2,
                                  space=bass.MemorySpace.PSUM) as pso:
                    for h in range(H):
                        dt, po = h // 2, (h % 2) * 64
                        eb = eb_p.tile([128, 16 * IB], BF16, tag="eb", name="eb")
                        nc.sync.dma_start(eb[:], expb_d[h])
                        ps_o = pso.tile([HD + 1, 512], F32, tag="ps_o",
                                        name="ps_o")
                        for j in range(16):
                            jb = slice(j * 128, (j + 1) * 128)
                            ps_s = pss.tile([128, 512], F32, tag="ps_s",
                                            name="ps_s")
                            nc.tensor.matmul(ps_s[:], kT[dt][po:po + 64, jb],
                                             qT[dt][po:po + 64, :],
                                             start=True, stop=True)
                            u = u_p.tile([128, IB], BF16, tag="u", name="u")
                            nc.scalar.activation(u[:], ps_s[:], AF.Exp)
                            ue = ue_p.tile([128, IB], BF16, tag="ue", name="ue")
                            nc.vector.tensor_mul(ue[:], u[:],
                                                 eb[:, j * IB:(j + 1) * IB])
                            nc.tensor.matmul(
                                ps_o[:],
                                vcat[j][:, h * (HD + 1):(h + 1) * (HD + 1)],
                                ue[:], start=(j == 0), stop=(j == 15))
                        recip = nrm_p.tile([1, 512], F32, tag="recip",
                                           name="recip")
                        nc.vector.reciprocal(recip[:], ps_o[64:65, :])
                        rb = nrm_p.tile([64, 512], F32, tag="rb", name="rb")
                        nc.gpsimd.partition_broadcast(rb[:], recip[:])
                        nc.vector.tensor_mul(oT[dt][po:po + 64, :],
                                             ps_o[0:64, :], rb[:])
            # kT/vcat/qT pools closed here

            # ===== Phase D: out-projection + LN2 ===========================
            with tc.tile_pool(name="wudma", bufs=1) as wu_p, \
                 tc.tile_pool(name="res", bufs=1) as res_p, \
                 tc.tile_pool(name="h2", bufs=1) as h2_p, \
                 tc.tile_pool(name="sz", bufs=1) as sz_p:
                wo = wu_p.tile([128, 8 * D], BF16, tag="wo", name="wo")
                nc.sync.dma_start(wo[:], wo_d[:])
                w1 = wu_p.tile([128, 8 * FF], BF16, tag="w1", name="w1")
                nc.sync.dma_start(w1[:], w1_d[:])
                # residual x block re-loaded here (cheaper than holding it
                # in SBUF through phases A-C)
                res = [res_p.tile([128, IB], F32, tag=f"res{e}",
                                  name=f"res{e}") for e in range(8)]
                rx = [res_p.tile([128, IB], F32, tag=f"rx{e}",
                                 name=f"rx{e}") for e in range(8)]
                for e in range(8):
                    nc.sync.dma_start(rx[e][:], xT_d[e * 128:(e + 1) * 128,
                                                     0:IB])
                h2 = [h2_p.tile([128, IB], BF16, tag=f"h2{e}", name=f"h2{e}")
                      for e in range(8)]
                sz = [sz_p.tile([128, IB], BF16, tag=f"sz{f}", name=f"sz{f}")
                      for f in range(32)]

                with tc.tile_pool(name="sq2", bufs=2) as sq2_p, \
                     tc.tile_pool(name="lnt2", bufs=6) as lnt2_p, \
                     tc.tile_pool(name="lnb2", bufs=2) as lnb2_p, \
                     tc.tile_pool(name="dps", bufs=2,
                                  space=bass.MemorySpace.PSUM) as dps, \
                     tc.tile_pool(name="dps1", bufs=2,
                                  space=bass.MemorySpace.PSUM) as dps1:
                    for et in range(8):
                        ps = dps.tile([128, 512], F32, tag="psx1", name="psx1")
                        for dt in range(8):
                            nc.tensor.matmul(
                                ps[:],
                                wo[:, dt * D + et * 128:dt * D + (et + 1) * 128],
                                oT[dt][:], start=(dt == 0), stop=(dt == 7))
                        # res[et] = x + attn_out (+bo2)
                        nc.vector.scalar_tensor_tensor(
                            res[et][:], ps[:], bo2[:, et:et + 1], rx[et][:],
                            op0=OP.add, op1=OP.add)
                    # LN2 (single 512-col block)
                    ps_mu = dps1.tile([1, 512], F32, tag="psmu2", name="psmu2")
                    for e in range(8):
                        nc.tensor.matmul(ps_mu[:], ones_f[:], res[e][:],
                                         start=(e == 0), stop=(e == 7))
                    ps_sq = dps1.tile([1, 512], F32, tag="pssq2", name="pssq2")
                    for e in range(8):
                        x2 = sq2_p.tile([128, 512], F32, tag="x22", name="x22")
                        nc.vector.tensor_mul(x2[:], res[e][:], res[e][:])
                        nc.tensor.matmul(ps_sq[:], ones_f[:], x2[:],
                                         start=(e == 0), stop=(e == 7))
                    mu_n = lnt2_p.tile([1, 512], F32, tag="stat2", name="mu_n2")
                    nc.scalar.activation(mu_n[:], ps_mu[:], AF.Identity,
                                         scale=1.0 / D)
                    mu2_n = lnt2_p.tile([1, 512], F32, tag="stat2",
                                        name="mu2_n2")
                    nc.scalar.square(mu2_n[:], mu_n[:])
                    var_n = lnt2_p.tile([1, 512], F32, tag="stat2",
                                        name="var_n2")
                    nc.vector.scalar_tensor_tensor(var_n[:], ps_sq[:], 1.0 / D,
                                                   mu2_n[:], op0=OP.mult,
                                                   op1=OP.subtract)
                    std_n = lnt2_p.tile([1, 512], F32, tag="stat2",
                                        name="std_n2")
                    nc.scalar.activation(std_n[:], var_n[:], AF.Sqrt,
                                         bias=eps_t[:])
                    rstd_n = lnt2_p.tile([1, 512], F32, tag="stat2",
                                         name="rstd_n2")
                    nc.vector.reciprocal(rstd_n[:], std_n[:])
                    mu_b = lnb2_p.tile([128, 512], F32, tag="mu_b2",
                                       name="mu_b2")
                    nc.gpsimd.partition_broadcast(mu_b[:], mu_n[:])
                    rstd_b = lnb2_p.tile([128, 512], F32, tag="rstd_b2",
                                         name="rstd_b2")
                    nc.gpsimd.partition_broadcast(rstd_b[:], rstd_n[:])
                    for e in range(8):
                        t = sq2_p.tile([128, IB], F32, tag="lnap2", name="lnap2")
                        nc.vector.tensor_sub(t[:], res[e][:], mu_b[:])
                        nc.vector.tensor_mul(t[:], t[:], rstd_b[:])
                        nc.vector.tensor_scalar(h2[e][:], t[:], g2[:, e:e + 1],
                                                bg2[:, e:e + 1],
                                                op0=OP.mult, op1=OP.add)

                # ===== Phase E: FFN ========================================
                # E2 runs ft-outer in two et-passes (et 0-3, then 4-7) with
                # 4-bank PSUM accumulation groups; W2 is streamed in
                # [128, 8*D] chunks (re-read once per pass).
                with tc.tile_pool(name="sg", bufs=3) as sg_p, \
                     tc.tile_pool(name="w2c", bufs=2) as w2c_p, \
                     tc.tile_pool(name="eps", bufs=3,
                                  space=bass.MemorySpace.PSUM) as eps_p, \
                     tc.tile_pool(name="e2ps", bufs=4,
                                  space=bass.MemorySpace.PSUM) as e2ps_p:
                    for ft in range(32):
                        ps = eps_p.tile([128, 512], F32, tag="pse", name="psz")
                        for e in range(8):
                            nc.tensor.matmul(
                                ps[:],
                                w1[:, e * FF + ft * 128:e * FF + (ft + 1) * 128],
                                h2[e][:], start=(e == 0), stop=(e == 7))
                        sg = sg_p.tile([128, IB], BF16, tag="sg", name="sg")
                        nc.scalar.activation(sg[:], ps[:], AF.Sigmoid,
                                             bias=b1[:, ft:ft + 1])
                        # silu(z) = z * sigmoid(z), z = ps + b1
                        nc.vector.scalar_tensor_tensor(sz[ft][:], ps[:],
                                                       b1[:, ft:ft + 1], sg[:],
                                                       op0=OP.add, op1=OP.mult)
                    for ep in range(2):  # et groups 0-3, 4-7
                        p2 = [e2ps_p.tile([128, 512], F32, tag="pse2",
                                          name="psy") for _ in range(4)]
                        for fc in range(4):  # W2 chunk of 8 ft-blocks
                            w2c = w2c_p.tile([128, 8 * D], BF16, tag="w2c",
                                             name="w2c")
                            nc.sync.dma_start(
                                w2c[:], w2_d[:, fc * 8 * D:(fc + 1) * 8 * D])
                            for fi in range(8):
                                ft = fc * 8 + fi
                                for ei in range(4):
                                    et = ep * 4 + ei
                                    nc.tensor.matmul(
                                        p2[ei][:],
                                        w2c[:, fi * D + et * 128:
                                             fi * D + (et + 1) * 128],
                                        sz[ft][:], start=(ft == 0),
                                        stop=(ft == 31))
                        for ei in range(4):
                            et = ep * 4 + ei
                            ot = out_p.tile([128, IB], F32, tag="outt",
                                            name="outt")
                            nc.vector.scalar_tensor_tensor(
                                ot[:], p2[ei][:], b2[:, et:et + 1],
                                res[et][:], op0=OP.add, op1=OP.add)
                            nc.sync.dma_start(
                                outT_d[et * 128:(et + 1) * 128, :], ot[:])

    nc.compile()
    return nc


def _prep_inputs(inputs):
    """Host-side layout prep -> list of 8 per-core input maps."""
    x = np.asarray(inputs["x"], dtype=np.float32)
    ab = np.asarray(inputs["attn_bias"], dtype=np.float32)

    def pack(v, ntiles):
        return np.ascontiguousarray(
            np.asarray(v, np.float32).reshape(ntiles, 128).T)

    def pack_w(w, nblk):
        # [nblk*128, cols] -> [128, nblk*cols]
        w = np.asarray(w, np.float32).astype(BF16_NP)
        cols = w.shape[1]
        return np.ascontiguousarray(
            w.reshape(nblk, 128, cols).transpose(1, 0, 2).reshape(
                128, nblk * cols))

    wq = pack_w(inputs["Wq"], 8)
    wk = pack_w(inputs["Wk"], 8)
    wv = pack_w(inputs["Wv"], 8)
    bo2 = (np.asarray(inputs["bo"], np.float32)
           + np.asarray(inputs["bv"], np.float32).astype(BF16_NP).astype(
               np.float32) @ np.asarray(inputs["Wo"], np.float32).astype(
                   BF16_NP).astype(np.float32))

    shared = {
        "wqkv": np.ascontiguousarray(np.concatenate([wq, wk, wv], axis=1)),
        "wo": pack_w(inputs["Wo"], 8),
        "w1": pack_w(inputs["W1"], 8),
        "w2": pack_w(inputs["W2"], 32),
        "g1": pack(inputs["ln1_g"], 8),
        "bg1": pack(inputs["ln1_b"], 8),
        "g2": pack(inputs["ln2_g"], 8),
        "bg2": pack(inputs["ln2_b"], 8),
        "bq8": pack(np.asarray(inputs["bq"], np.float32) * SCALE, 8),
        "bk": pack(inputs["bk"], 8),
        "bo2": pack(bo2, 8),
        "b1": pack(inputs["b1"], 32),
        "b2": pack(inputs["b2"], 8),
    }
    xT = [np.ascontiguousarray(x[b].T) for b in range(B)]  # [D, T] f32
    in_maps = []
    for b in range(B):
        # exp(bias) for this batch, bf16: [H, T_i, T_j]
        eb_b = np.exp(ab[b]).astype(BF16_NP)
        for ci in range(4):
            i0 = ci * IB
            # token axis rotated by -i0 (queries land at cols 0:IB); the j
            # axis of the bias is rotated identically to match k/v order.
            xTc = np.ascontiguousarray(np.roll(xT[b], -i0, axis=1))
            ebs = np.roll(eb_b[:, i0:i0 + IB, :], -i0, axis=2)  # [H, IB, T]
            # -> [H, 128, 16*IB]: [h, p, jb*IB+i] = ebs[h, i, jb*128+p]
            ebs = np.ascontiguousarray(
                ebs.transpose(0, 2, 1).reshape(H, 16, 128, IB)
                .transpose(0, 2, 1, 3).reshape(H, 128, 16 * IB))
            m = {"xT": xTc, "expb": ebs}
            m.update(shared)
            in_maps.append(m)
    return in_maps


def kernel(**inputs):
    if "nc" not in _cache:
        _cache["nc"] = build_program()
    nc = _cache["nc"]
    in_maps = _prep_inputs(inputs)
    r = run_bass_kernel_spmd(nc, in_maps, list(range(N_CORES)))
    out = np.empty((B, T, D), dtype=np.float32)
    for c in range(N_CORES):
        b, i0 = c // 4, (c % 4) * IB
        out[b, i0:i0 + IB, :] = np.asarray(r.results[c]["outT"], np.float32).T
    return out


# revision 3
# speedup vs baseline: 2.5927x; 2.2355x over previous
"""Graphormer layer (pre-norm MHSA + additive attn bias + SiLU FFN) on 8 trn2 cores.

Sharding: core c handles batch b = c//4 and query rows i0 = (c%4)*512 .. +512.
Each core computes LN1 + full K/V for its batch (replicated inside the
4-core batch group), Q/scores/softmax/attn@V for its 512 query rows, the
output projection, LN2 and the full FFN for those rows.  No collectives.

v2 changes vs baseline (which was DMA-dispatch-bound at ~600ns/dma on
SP.SEQ + HWDGE):
  - ~50 large DMAs instead of ~1200 small ones: weights host-packed into
    [128, N] row-block layouts loaded in 1 DMA each; attn bias host-packed
    per head as [128, 16*512] tiles (16 DMAs).
  - host precomputes exp(bias); device computes u = exp(scores) straight
    from PSUM on ACT, then one fast bf16 multiply u*exp(bias) on DVE
    (replaces the f32 bias-add + exp roundtrip).
  - bv folded into bo on host (softmax rows sum to 1 so (v+bv)@Wo =
    v@Wo + bv@Wo), removing the ones-row matmuls in the V projection.
  - loop orders reuse the matmul stationary operand (K proj: 1 ldweights
    per 4 matmuls) and keep PSUM accumulation groups within 8 banks.

Layouts on device are feature-major ("transposed"): xT [D, T], qT/kT [d, T],
scoresT [j, i].  The softmax denominator comes from appending a ones column
to V in the attn@V matmul; normalization uses a gpsimd partition-broadcast
of the reciprocal.  Matmul operands are bf16 (fp32 accumulation in PSUM);
the residual path stays fp32.  Softmax skips the max-subtraction: scores
are O(8) here so exp stays comfortably inside fp32/bf16 range.
"""

import sys
from contextlib import ExitStack

import numpy as np

sys.path.insert(0, "/opt/trn_rl_repo")

import ml_dtypes  # noqa: E402

import concourse.bass as bass  # noqa: E402
import concourse.bacc as bacc  # noqa: E402
import concourse.tile as tile  # noqa: E402
from concourse import mybir  # noqa: E402
from concourse.bass_utils import run_bass_kernel_spmd  # noqa: E402

F32 = mybir.dt.float32
BF16 = mybir.dt.bfloat16
FP8 = mybir.dt.float8e4
DR = mybir.MatmulPerfMode.DoubleRow
AF = mybir.ActivationFunctionType
OP = mybir.AluOpType
BF16_NP = ml_dtypes.bfloat16
FP8_NP = ml_dtypes.float8_e4m3

B, T, D = 2, 2048, 1024
H, HD = 16, 64
FF = 4 * D
N_CORES = 8
IB = 512           # query rows per core
SCALE = 1.0 / 8.0  # 1/sqrt(HD)
EPS = 1e-5

_cache = {}


def build_program():
    nc = bacc.Bacc("TRN2", target_bir_lowering=False, debug=False)

    # ---- DRAM I/O ----
    xT_d = nc.dram_tensor("xT", [D, T], F32, kind="ExternalInput").ap()
    # exp(bias) per head: [H, 128, 16*512]; [h, p, jb*512+i] = exp(b[h, i, jb*128+p])
    expb_d = nc.dram_tensor("expb", [H, 128, 16 * IB], BF16, kind="ExternalInput").ap()
    # weights packed as [128, ...] row-block layouts (see _prep_inputs)
    wqkv_d = nc.dram_tensor("wqkv", [128, 3 * 8 * D], FP8, kind="ExternalInput").ap()
    wo_d = nc.dram_tensor("wo", [128, 8 * D], BF16, kind="ExternalInput").ap()
    w1_d = nc.dram_tensor("w1", [128, 8 * FF], BF16, kind="ExternalInput").ap()
    w2_d = nc.dram_tensor("w2", [128, 32 * D], BF16, kind="ExternalInput").ap()
    # packed per-partition params: [128, n_tiles] fp32
    g1_d = nc.dram_tensor("g1", [128, 8], F32, kind="ExternalInput").ap()
    bg1_d = nc.dram_tensor("bg1", [128, 8], F32, kind="ExternalInput").ap()
    g2_d = nc.dram_tensor("g2", [128, 8], F32, kind="ExternalInput").ap()
    bg2_d = nc.dram_tensor("bg2", [128, 8], F32, kind="ExternalInput").ap()
    bq8_d = nc.dram_tensor("bq8", [128, 8], F32, kind="ExternalInput").ap()
    bk_d = nc.dram_tensor("bk", [128, 8], F32, kind="ExternalInput").ap()
    bo2_d = nc.dram_tensor("bo2", [128, 8], F32, kind="ExternalInput").ap()
    b1_d = nc.dram_tensor("b1", [128, 32], F32, kind="ExternalInput").ap()
    b2_d = nc.dram_tensor("b2", [128, 8], F32, kind="ExternalInput").ap()
    outT_d = nc.dram_tensor("outT", [D, IB], F32, kind="ExternalOutput").ap()

    with tile.TileContext(nc) as tc, ExitStack() as ctx:
        # ---------------- outermost (whole-kernel lifetime) ----------------
        const_p = ctx.enter_context(tc.tile_pool(name="const", bufs=1))
        param_p = ctx.enter_context(tc.tile_pool(name="param", bufs=1))
        out_p = ctx.enter_context(tc.tile_pool(name="out", bufs=2))

        ones_f = const_p.tile([128, 1], F32, tag="ones_f")
        nc.vector.memset(ones_f[:], 1.0)
        eps_t = const_p.tile([1, 1], F32, tag="eps")
        nc.vector.memset(eps_t[:], EPS)

        def load_param(name, dram, shape, dtype=F32):
            t = param_p.tile(shape, dtype, tag=name, name=name)
            nc.sync.dma_start(t[:], dram[:])
            return t

        g1 = load_param("g1", g1_d, [128, 8])
        bg1 = load_param("bg1", bg1_d, [128, 8])
        g2 = load_param("g2", g2_d, [128, 8])
        bg2 = load_param("bg2", bg2_d, [128, 8])
        bq8 = load_param("bq8", bq8_d, [128, 8])
        bk = load_param("bk", bk_d, [128, 8])
        bo2 = load_param("bo2", bo2_d, [128, 8])
        b1 = load_param("b1", b1_d, [128, 32])
        b2 = load_param("b2", b2_d, [128, 8])

        # ---------------- scope: oT (phases C-D) ---------------------------
        with tc.tile_pool(name="oT", bufs=1) as oT_p:
            oT = [oT_p.tile([128, IB], BF16, tag=f"oT{d}", name=f"oT{d}")
                  for d in range(8)]

            # ------------- scope: K/V/Q (phases A-C) -----------------------
            with tc.tile_pool(name="kT", bufs=1) as kT_p, \
                 tc.tile_pool(name="vcat", bufs=1) as vcat_p, \
                 tc.tile_pool(name="qT", bufs=1) as qT_p:
                kT = [kT_p.tile([128, T], BF16, tag=f"kT{d}", name=f"kT{d}")
                      for d in range(8)]
                vcat = [vcat_p.tile([128, H * (HD + 1)], BF16, tag=f"vc{t}",
                                    name=f"vc{t}") for t in range(16)]
                qT = [qT_p.tile([128, IB], BF16, tag=f"qT{d}", name=f"qT{d}")
                      for d in range(8)]

                # ------------- scope: hT + wqkv (phases A-B) ---------------
                with tc.tile_pool(name="hT", bufs=1) as hT_p, \
                     tc.tile_pool(name="wqk", bufs=1) as wqk_p:
                    hTp = [hT_p.tile([128, 2 * T], FP8, tag=f"hTp{ep}",
                                     name=f"hTp{ep}") for ep in range(4)]

                    def h_mv(ep, cols):
                        # moving-operand pair view [128, 2, len(cols)]
                        return hTp[ep][:].rearrange(
                            "p (two t) -> p two t", two=2)[:, :, cols]

                    wqk = wqk_p.tile([128, 2 * 8 * D], FP8, tag="wqk",
                                     name="wqk")
                    nc.sync.dma_start(wqk[:], wqkv_d[:, 0:2 * 8 * D])

                    def w_pair(tile, base, ep, cols):
                        # stationary pair view [128, 2, len(cols)]
                        return tile[:, base + 2 * ep * D:
                                    base + (2 * ep + 2) * D].rearrange(
                            "p (two d) -> p two d", two=2)[:, :, cols]

                    # ===== Phase A: LN1 (streamed, partition-axis stats) ===
                    with tc.tile_pool(name="xc", bufs=10) as xc_p, \
                         tc.tile_pool(name="sq", bufs=2) as sq_p, \
                         tc.tile_pool(name="lnt", bufs=6) as lnt_p, \
                         tc.tile_pool(name="lnb", bufs=2) as lnb_p, \
                         tc.tile_pool(name="lnps", bufs=4,
                                      space=bass.MemorySpace.PSUM) as lnps_p:
                        for n in range(4):
                            nb = slice(n * 512, (n + 1) * 512)
                            xcs = []
                            for e in range(8):
                                xc = xc_p.tile([128, 512], F32, tag="xc",
                                               name="xc")
                                nc.sync.dma_start(
                                    xc[:], xT_d[e * 128:(e + 1) * 128, nb])
                                xcs.append(xc)
                            ps_mu = lnps_p.tile([1, 512], F32, tag="psmu",
                                                name="psmu")
                            ps_sq = lnps_p.tile([1, 512], F32, tag="pssq",
                                                name="pssq")
                            for e in range(8):
                                nc.tensor.matmul(ps_mu[:], ones_f[:],
                                                 xcs[e][:],
                                                 start=(e == 0), stop=(e == 7))
                            for e in range(8):
                                x2 = sq_p.tile([128, 512], F32, tag="x2",
                                               name="x2")
                                nc.vector.tensor_mul(x2[:], xcs[e][:],
                                                     xcs[e][:])
                                nc.tensor.matmul(ps_sq[:], ones_f[:], x2[:],
                                                 start=(e == 0), stop=(e == 7))
                            mu_n = lnt_p.tile([1, 512], F32, tag="stat",
                                              name="mu_n")
                            nc.scalar.activation(mu_n[:], ps_mu[:], AF.Identity,
                                                 scale=1.0 / D)
                            mu2_n = lnt_p.tile([1, 512], F32, tag="stat",
                                               name="mu2_n")
                            nc.scalar.square(mu2_n[:], mu_n[:])
                            var_n = lnt_p.tile([1, 512], F32, tag="stat",
                                               name="var_n")
                            nc.vector.scalar_tensor_tensor(
                                var_n[:], ps_sq[:], 1.0 / D, mu2_n[:],
                                op0=OP.mult, op1=OP.subtract)
                            std_n = lnt_p.tile([1, 512], F32, tag="stat",
                                               name="std_n")
                            nc.scalar.activation(std_n[:], var_n[:], AF.Sqrt,
                                                 bias=eps_t[:])
                            rstd_n = lnt_p.tile([1, 512], F32, tag="stat",
                                                name="rstd_n")
                            nc.vector.reciprocal(rstd_n[:], std_n[:])
                            mu_b = lnb_p.tile([128, 512], F32, tag="mu_b",
                                              name="mu_b")
                            nc.gpsimd.partition_broadcast(mu_b[:], mu_n[:])
                            rstd_b = lnb_p.tile([128, 512], F32, tag="rstd_b",
                                                name="rstd_b")
                            nc.gpsimd.partition_broadcast(rstd_b[:], rstd_n[:])
                            for e in range(8):
                                t = sq_p.tile([128, 512], F32, tag="lnap",
                                              name="lnap")
                                nc.vector.tensor_sub(t[:], xcs[e][:], mu_b[:])
                                nc.vector.tensor_mul(t[:], t[:], rstd_b[:])
                                nc.vector.tensor_scalar(
                                    hTp[e // 2][:, (e % 2) * T + n * 512:
                                                (e % 2) * T + (n + 1) * 512],
                                    t[:], g1[:, e:e + 1],
                                    bg1[:, e:e + 1], op0=OP.mult, op1=OP.add)

                    # ===== Phase B: K/Q/V projections ======================
                    with tc.tile_pool(name="wv", bufs=1) as wv_p, \
                         tc.tile_pool(name="pps", bufs=8,
                                      space=bass.MemorySpace.PSUM) as pps:
                        wv = wv_p.tile([128, 8 * D], FP8, tag="wv", name="wv")
                        nc.sync.dma_start(wv[:],
                                          wqkv_d[:, 2 * 8 * D:3 * 8 * D])
                        # kT[d, j] over all tokens; stationary reused over n
                        for dt in range(8):
                            pk = [pps.tile([128, 512], F32, tag="ps", name="psk")
                                  for _ in range(4)]
                            for ep in range(4):
                                wsl = w_pair(wqk, 8 * D, ep,
                                             slice(dt * 128, (dt + 1) * 128))
                                for n in range(4):
                                    nc.tensor.matmul(
                                        pk[n][:], wsl,
                                        h_mv(ep, slice(n * 512, (n + 1) * 512)),
                                        start=(ep == 0), stop=(ep == 3),
                                        perf_mode=DR)
                            for n in range(4):
                                nc.vector.tensor_scalar_add(
                                    kT[dt][:, n * 512:(n + 1) * 512], pk[n][:],
                                    bk[:, dt:dt + 1])
                        # qT[d, i] for this core's rows (= token cols 0:IB)
                        for dt in range(8):
                            ps = pps.tile([128, 512], F32, tag="ps", name="psq")
                            for ep in range(4):
                                nc.tensor.matmul(
                                    ps[:],
                                    w_pair(wqk, 0, ep,
                                           slice(dt * 128, (dt + 1) * 128)),
                                    h_mv(ep, slice(0, IB)),
                                    start=(ep == 0), stop=(ep == 3),
                                    perf_mode=DR)
                            nc.scalar.activation(qT[dt][:], ps[:], AF.Identity,
                                                 scale=SCALE,
                                                 bias=bq8[:, dt:dt + 1])
                        # v[j, d] natural layout + ones column per head
                        for tt in range(16):
                            nc.vector.memset(
                                vcat[tt][:].rearrange(
                                    "p (h x) -> p h x", x=HD + 1)[:, :,
                                                                  HD:HD + 1],
                                1.0)
                        for tt in range(16):
                            tb = slice(tt * 128, (tt + 1) * 128)
                            pv = [pps.tile([128, 512], F32, tag="ps", name="psv")
                                  for _ in range(2)]
                            for ep in range(4):
                                hsl = h_mv(ep, tb)
                                for n in range(2):
                                    nc.tensor.matmul(
                                        pv[n][:], hsl,
                                        w_pair(wv, 0, ep,
                                               slice(n * 512, (n + 1) * 512)),
                                        start=(ep == 0), stop=(ep == 3),
                                        perf_mode=DR)
                            for n in range(2):
                                dst = vcat[tt][:, n * 8 * (HD + 1):
                                               (n + 1) * 8 * (HD + 1)]
                                dst = dst.rearrange("p (h x) -> p h x",
                                                    x=HD + 1)[:, :, 0:HD]
                                src = pv[n][:].rearrange("p (h d) -> p h d",
                                                         d=HD)
                                nc.scalar.activation(dst, src, AF.Identity)
                # hT/wqkv pools closed here

                # ===== Phase C: attention ==================================
                with tc.tile_pool(name="ebias", bufs=3) as eb_p, \
                     tc.tile_pool(name="uexp", bufs=4) as u_p, \
                     tc.tile_pool(name="uexb", bufs=4) as ue_p, \
                     tc.tile_pool(name="nrm", bufs=2) as nrm_p, \
                     tc.tile_pool(name="pss", bufs=3,
                                  space=bass.MemorySpace.PSUM) as pss, \
                     tc.tile_pool(name="pso", bufs=2,
                                  space=bass.MemorySpace.PSUM) as pso:
                    for h in range(H):
                        dt, po = h // 2, (h % 2) * 64
                        eb = eb_p.tile([128, 16 * IB], BF16, tag="eb", name="eb")
                        nc.sync.dma_start(eb[:], expb_d[h])
                        ps_o = pso.tile([HD + 1, 512], F32, tag="ps_o",
                                        name="ps_o")
                        for j in range(16):
                            jb = slice(j * 128, (j + 1) * 128)
                            ps_s = pss.tile([128, 512], F32, tag="ps_s",
                                            name="ps_s")
                            nc.tensor.matmul(ps_s[:], kT[dt][po:po + 64, jb],
                                             qT[dt][po:po + 64, :],
                                             start=True, stop=True)
                            u = u_p.tile([128, IB], BF16, tag="u", name="u")
                            nc.scalar.activation(u[:], ps_s[:], AF.Exp)
                            ue = ue_p.tile([128, IB], BF16, tag="ue", name="ue")
                            nc.vector.tensor_mul(ue[:], u[:],
                                                 eb[:, j * IB:(j + 1) * IB])
                            nc.tensor.matmul(
                                ps_o[:],
                                vcat[j][:, h * (HD + 1):(h + 1) * (HD + 1)],
                                ue[:], start=(j == 0), stop=(j == 15))
                        recip = nrm_p.tile([1, 512], F32, tag="recip",
                                           name="recip")
                        nc.vector.reciprocal(recip[:], ps_o[64:65, :])
                        rb = nrm_p.tile([64, 512], F32, tag="rb", name="rb")
                        nc.gpsimd.partition_broadcast(rb[:], recip[:])
                        nc.vector.tensor_mul(oT[dt][po:po + 64, :],
                                             ps_o[0:64, :], rb[:])
            # kT/vcat/qT pools closed here

            # ===== Phase D: out-projection + LN2 ===========================
            with tc.tile_pool(name="wudma", bufs=1) as wu_p, \
                 tc.tile_pool(name="res", bufs=1) as res_p, \
                 tc.tile_pool(name="h2", bufs=1) as h2_p, \
                 tc.tile_pool(name="sz", bufs=1) as sz_p:
                wo = wu_p.tile([128, 8 * D], BF16, tag="wo", name="wo")
                nc.sync.dma_start(wo[:], wo_d[:])
                w1 = wu_p.tile([128, 8 * FF], BF16, tag="w1", name="w1")
                nc.sync.dma_start(w1[:], w1_d[:])
                # residual x block re-loaded here (cheaper than holding it
                # in SBUF through phases A-C)
                res = [res_p.tile([128, IB], F32, tag=f"res{e}",
                                  name=f"res{e}") for e in range(8)]
                rx = [res_p.tile([128, IB], F32, tag=f"rx{e}",
                                 name=f"rx{e}") for e in range(8)]
                for e in range(8):
                    nc.sync.dma_start(rx[e][:], xT_d[e * 128:(e + 1) * 128,
                                                     0:IB])
                h2 = [h2_p.tile([128, IB], BF16, tag=f"h2{e}", name=f"h2{e}")
                      for e in range(8)]
                sz = [sz_p.tile([128, IB], BF16, tag=f"sz{f}", name=f"sz{f}")
                      for f in range(32)]

                with tc.tile_pool(name="sq2", bufs=2) as sq2_p, \
                     tc.tile_pool(name="lnt2", bufs=6) as lnt2_p, \
                     tc.tile_pool(name="lnb2", bufs=2) as lnb2_p, \
                     tc.tile_pool(name="dps", bufs=2,
                                  space=bass.MemorySpace.PSUM) as dps, \
                     tc.tile_pool(name="dps1", bufs=2,
                                  space=bass.MemorySpace.PSUM) as dps1:
                    for et in range(8):
                        ps = dps.tile([128, 512], F32, tag="psx1", name="psx1")
                        for dt in range(8):
                            nc.tensor.matmul(
                                ps[:],
                                wo[:, dt * D + et * 128:dt * D + (et + 1) * 128],
                                oT[dt][:], start=(dt == 0), stop=(dt == 7))
                        # res[et] = x + attn_out (+bo2)
                        nc.vector.scalar_tensor_tensor(
                            res[et][:], ps[:], bo2[:, et:et + 1], rx[et][:],
                            op0=OP.add, op1=OP.add)
                    # LN2 (single 512-col block)
                    ps_mu = dps1.tile([1, 512], F32, tag="psmu2", name="psmu2")
                    for e in range(8):
                        nc.tensor.matmul(ps_mu[:], ones_f[:], res[e][:],
                                         start=(e == 0), stop=(e == 7))
                    ps_sq = dps1.tile([1, 512], F32, tag="pssq2", name="pssq2")
                    for e in range(8):
                        x2 = sq2_p.tile([128, 512], F32, tag="x22", name="x22")
                        nc.vector.tensor_mul(x2[:], res[e][:], res[e][:])
                        nc.tensor.matmul(ps_sq[:], ones_f[:], x2[:],
                                         start=(e == 0), stop=(e == 7))
                    mu_n = lnt2_p.tile([1, 512], F32, tag="stat2", name="mu_n2")
                    nc.scalar.activation(mu_n[:], ps_mu[:], AF.Identity,
                                         scale=1.0 / D)
                    mu2_n = lnt2_p.tile([1, 512], F32, tag="stat2",
                                        name="mu2_n2")
                    nc.scalar.square(mu2_n[:], mu_n[:])
                    var_n = lnt2_p.tile([1, 512], F32, tag="stat2",
                                        name="var_n2")
                    nc.vector.scalar_tensor_tensor(var_n[:], ps_sq[:], 1.0 / D,
                                                   mu2_n[:], op0=OP.mult,
                                                   op1=OP.subtract)
                    std_n = lnt2_p.tile([1, 512], F32, tag="stat2",
                                        name="std_n2")
                    nc.scalar.activation(std_n[:], var_n[:], AF.Sqrt,
                                         bias=eps_t[:])
                    rstd_n = lnt2_p.tile([1, 512], F32, tag="stat2",
                                         name="rstd_n2")
                    nc.vector.reciprocal(rstd_n[:], std_n[:])
                    mu_b = lnb2_p.tile([128, 512], F32, tag="mu_b2",
                                       name="mu_b2")
                    nc.gpsimd.partition_broadcast(mu_b[:], mu_n[:])
                    rstd_b = lnb2_p.tile([128, 512], F32, tag="rstd_b2",
                                         name="rstd_b2")
                    nc.gpsimd.partition_broadcast(rstd_b[:], rstd_n[:])
                    for e in range(8):
                        t = sq2_p.tile([128, IB], F32, tag="lnap2", name="lnap2")
                        nc.vector.tensor_sub(t[:], res[e][:], mu_b[:])
                        nc.vector.tensor_mul(t[:], t[:], rstd_b[:])
                        nc.vector.tensor_scalar(h2[e][:], t[:], g2[:, e:e + 1],
                                                bg2[:, e:e + 1],
                                                op0=OP.mult, op1=OP.add)

                # ===== Phase E: FFN ========================================
                # E2 runs ft-outer in two et-passes (et 0-3, then 4-7) with
                # 4-bank PSUM accumulation groups; W2 is streamed in
                # [128, 8*D] chunks (re-read once per pass).
                with tc.tile_pool(name="sg", bufs=3) as sg_p, \
                     tc.tile_pool(name="w2c", bufs=2) as w2c_p, \
                     tc.tile_pool(name="eps", bufs=3,
                                  space=bass.MemorySpace.PSUM) as eps_p, \
                     tc.tile_pool(name="e2ps", bufs=4,
                                  space=bass.MemorySpace.PSUM) as e2ps_p:
                    for ft in range(32):
                        ps = eps_p.tile([128, 512], F32, tag="pse", name="psz")
                        for e in range(8):
                            nc.tensor.matmul(
                                ps[:],
                                w1[:, e * FF + ft * 128:e * FF + (ft + 1) * 128],
                                h2[e][:], start=(e == 0), stop=(e == 7))
                        sg = sg_p.tile([128, IB], BF16, tag="sg", name="sg")
                        nc.scalar.activation(sg[:], ps[:], AF.Sigmoid,
                                             bias=b1[:, ft:ft + 1])
                        # silu(z) = z * sigmoid(z), z = ps + b1
                        nc.vector.scalar_tensor_tensor(sz[ft][:], ps[:],
                                                       b1[:, ft:ft + 1], sg[:],
                                                       op0=OP.add, op1=OP.mult)
                    for ep in range(2):  # et groups 0-3, 4-7
                        p2 = [e2ps_p.tile([128, 512], F32, tag="pse2",
                                          name="psy") for _ in range(4)]
                        for fc in range(4):  # W2 chunk of 8 ft-blocks
                            w2c = w2c_p.tile([128, 8 * D], BF16, tag="w2c",
                                             name="w2c")
                            nc.sync.dma_start(
                                w2c[:], w2_d[:, fc * 8 * D:(fc + 1) * 8 * D])
                            for fi in range(8):
                                ft = fc * 8 + fi
                                for ei in range(4):
                                    et = ep * 4 + ei
                                    nc.tensor.matmul(
                                        p2[ei][:],
                                        w2c[:, fi * D + et * 128:
                                             fi * D + (et + 1) * 128],
                                        sz[ft][:], start=(ft == 0),
                                        stop=(ft == 31))
                        for ei in range(4):
                            et = ep * 4 + ei
                            ot = out_p.tile([128, IB], F32, tag="outt",
                                            name="outt")
                            nc.vector.scalar_tensor_tensor(
                                ot[:], p2[ei][:], b2[:, et:et + 1],
                                res[et][:], op0=OP.add, op1=OP.add)
                            nc.sync.dma_start(
                                outT_d[et * 128:(et + 1) * 128, :], ot[:])

    nc.compile()
    return nc


def _prep_inputs(inputs):
    """Host-side layout prep -> list of 8 per-core input maps."""
    x = np.asarray(inputs["x"], dtype=np.float32)
    ab = np.asarray(inputs["attn_bias"], dtype=np.float32)

    def pack(v, ntiles):
        return np.ascontiguousarray(
            np.asarray(v, np.float32).reshape(ntiles, 128).T)

    def pack_w(w, nblk, dtype=BF16_NP):
        # [nblk*128, cols] -> [128, nblk*cols]
        w = np.asarray(w, np.float32).astype(dtype)
        cols = w.shape[1]
        return np.ascontiguousarray(
            w.reshape(nblk, 128, cols).transpose(1, 0, 2).reshape(
                128, nblk * cols))

    wq = pack_w(inputs["Wq"], 8, FP8_NP)
    wk = pack_w(inputs["Wk"], 8, FP8_NP)
    wv = pack_w(inputs["Wv"], 8, FP8_NP)
    bo2 = (np.asarray(inputs["bo"], np.float32)
           + np.asarray(inputs["bv"], np.float32).astype(BF16_NP).astype(
               np.float32) @ np.asarray(inputs["Wo"], np.float32).astype(
                   BF16_NP).astype(np.float32))

    shared = {
        "wqkv": np.ascontiguousarray(np.concatenate([wq, wk, wv], axis=1)),
        "wo": pack_w(inputs["Wo"], 8),
        "w1": pack_w(inputs["W1"], 8),
        "w2": pack_w(inputs["W2"], 32),
        "g1": pack(inputs["ln1_g"], 8),
        "bg1": pack(inputs["ln1_b"], 8),
        "g2": pack(inputs["ln2_g"], 8),
        "bg2": pack(inputs["ln2_b"], 8),
        "bq8": pack(np.asarray(inputs["bq"], np.float32) * SCALE, 8),
        "bk": pack(inputs["bk"], 8),
        "bo2": pack(bo2, 8),
        "b1": pack(inputs["b1"], 32),
        "b2": pack(inputs["b2"], 8),
    }
    xT = [np.ascontiguousarray(x[b].T) for b in range(B)]  # [D, T] f32
    in_maps = []
    for b in range(B):
        # exp(bias) for this batch, bf16: [H, T_i, T_j]
        eb_b = np.exp(ab[b]).astype(BF16_NP)
        for ci in range(4):
            i0 = ci * IB
            # token axis rotated by -i0 (queries land at cols 0:IB); the j
            # axis of the bias is rotated identically to match k/v order.
            xTc = np.ascontiguousarray(np.roll(xT[b], -i0, axis=1))
            ebs = np.roll(eb_b[:, i0:i0 + IB, :], -i0, axis=2)  # [H, IB, T]
            # -> [H, 128, 16*IB]: [h, p, jb*IB+i] = ebs[h, i, jb*128+p]
            ebs = np.ascontiguousarray(
                ebs.transpose(0, 2, 1).reshape(H, 16, 128, IB)
                .transpose(0, 2, 1, 3).reshape(H, 128, 16 * IB))
            m = {"xT": xTc, "expb": ebs}
            m.update(shared)
            in_maps.append(m)
    return in_maps


def kernel(**inputs):
    if "nc" not in _cache:
        _cache["nc"] = build_program()
    nc = _cache["nc"]
    in_maps = _prep_inputs(inputs)
    r = run_bass_kernel_spmd(nc, in_maps, list(range(N_CORES)))
    out = np.empty((B, T, D), dtype=np.float32)
    for c in range(N_CORES):
        b, i0 = c // 4, (c % 4) * IB
        out[b, i0:i0 + IB, :] = np.asarray(r.results[c]["outT"], np.float32).T
    return out
